# revision 13
# baseline (speedup 1.0000x reference)
"""Trainium2 Bass kernel for nn_LinearSelfAttention (linear attention w/ RoPE,
elu+1 feature map, qkv + out projections).

Sharding: 8 cores = 4 batches x 2 head-groups (8 heads each).
Each core computes, for its (batch b, head-group g):
  qkv slice projection, RoPE, feature maps, per-head kv state (64x64),
  attention output, and a partial out-projection (its heads' rows of W_out).
Host sums the two head-group partials per batch and transposes.

Layout strategy (everything feature-major where matmuls need it):
  - host passes xT = x[b].T (D=1024, T=4096)
  - q is produced in (d, t) layout (W_q stationary, xT moving)
  - k, v are produced in (t, d) layout (xT tiles stationary, W_kv moving)
  - head-dims are interleaved (d, d+32) -> (2j, 2j+1) by permuting W_q/W_k
    columns on the host so the rotate-half partner is the XOR-1 partition
    (reachable by DVE stream_shuffle) / XOR-1 free element.
  - all large matmuls run as float32r (fast fp32 mode, ~1e-4 rel err)
"""

import sys

sys.path.insert(0, "/opt/trn_rl_repo")

import ml_dtypes
import numpy as np

import concourse.bacc as bacc
import concourse.mybir as mybir
from concourse.tile import TileContext
from concourse.bass_utils import run_bass_kernel_spmd

FP32 = mybir.dt.float32
FP32R = mybir.dt.float32r
F16 = mybir.dt.float16
AF = mybir.ActivationFunctionType
ALU = mybir.AluOpType

DIM = 1024
T = 4096
HEADS = 16
HD = 64
H_CORE = 8  # heads per core
ROPE_BASE = 500000.0
SCALE = HD**-0.5

NCH = 8  # chunks over T
CH = T // NCH  # 512 tokens per chunk
NSUB = CH // 128  # 4 sub-chunks of 128 tokens
DC = DIM // 128  # 8 contraction tiles
NPAIR = H_CORE // 2  # 4 head pairs (128 dims each)

_SHUF_MASK = [i ^ 1 for i in range(32)]


def _build():
    nc = bacc.Bacc(None, target_bir_lowering=False, debug=False)

    xT = nc.declare_dram_parameter("xT", [DIM, T], F16, isOutput=False)
    wq = nc.declare_dram_parameter("wq", [DIM, 512], F16, isOutput=False)
    wkv = nc.declare_dram_parameter("wkv", [DIM, 1024], F16, isOutput=False)
    wo = nc.declare_dram_parameter("wo", [512, DIM], F16, isOutput=False)
    cosq = nc.declare_dram_parameter("cosq", [128, T], FP32, isOutput=False)
    sinq = nc.declare_dram_parameter("sinq", [128, T], FP32, isOutput=False)
    cosk = nc.declare_dram_parameter("cosk", [128, 32 * HD], FP32, isOutput=False)
    sink = nc.declare_dram_parameter("sink", [128, 32 * HD], FP32, isOutput=False)
    sel = nc.declare_dram_parameter("sel", [8, 512], FP32, isOutput=False)
    ones16 = nc.declare_dram_parameter("ones16", [128, 4], F16, isOutput=False)
    zpad = nc.declare_dram_parameter("zpad", [128, 128], FP32, isOutput=False)
    outT = nc.declare_dram_parameter("outT", [DIM, T], FP32, isOutput=True)

    with TileContext(nc) as tc, nc.allow_low_precision(
        reason="fp32r tiles feed fp32r matmuls; rounding is intended"
    ):
        with tc.tile_pool(name="persist", bufs=1) as persist:
            # resident across both phases
            qf = [persist.tile([128, T], FP32R, tag=f"qf{p}", name=f"qf{p}") for p in range(NPAIR)]
            bdiag = [
                persist.tile([128, 128], FP32R, tag=f"bd{p}", name=f"bd{p}")
                for p in range(NPAIR)
            ]
            den_l = [
                persist.tile([128, 8], FP32R, tag=f"dl{p}", name=f"dl{p}")
                for p in range(NPAIR)
            ]

            wo_t = []
            for p in range(NPAIR):
                t_ = persist.tile([128, 1024], F16, tag=f"wo{p}", name=f"wo{p}")
                wo_t.append(t_)
            sel_t = persist.tile([8, 512], FP32R, tag="sel")

            with tc.tile_pool(name="pskv", bufs=1, space="PSUM") as pskv:
                kvps = [pskv.tile([128, 258], FP32, tag=f"kv{p}", name=f"kv{p}") for p in range(NPAIR)]

                # ---------------- phase 1 ----------------
                with tc.tile_pool(name="w1", bufs=1) as w1, tc.tile_pool(
                    name="s1", bufs=2
                ) as s1, tc.tile_pool(name="ps1", bufs=1, space="PSUM") as ps1:
                    wq_t = []
                    for dc in range(DC):
                        t_ = w1.tile([128, 512], F16, tag=f"wq{dc}", name=f"wq{dc}")
                        nc.sync.dma_start(
                            out=t_[:], in_=wq[dc * 128 : (dc + 1) * 128, :]
                        )
                        wq_t.append(t_)
                    wkv_t = []
                    for dc in range(DC):
                        t_ = w1.tile([128, 1024], F16, tag=f"wkv{dc}", name=f"wkv{dc}")
                        nc.sync.dma_start(
                            out=t_[:], in_=wkv[dc * 128 : (dc + 1) * 128, :]
                        )
                        wkv_t.append(t_)
                    for p in range(NPAIR):
                        nc.sync.dma_start(
                            out=wo_t[p][:], in_=wo[p * 128 : (p + 1) * 128, :]
                        )
                    nc.sync.dma_start(out=sel_t[:], in_=sel[:].bitcast(FP32R))

                    for c in range(NCH):
                        tsl = slice(c * CH, (c + 1) * CH)
                        xt = []
                        for dc in range(DC):
                            t_ = s1.tile([128, CH], F16, tag=f"x{dc}", name=f"x{dc}")
                            nc.sync.dma_start(
                                out=t_[:], in_=xT[dc * 128 : (dc + 1) * 128, tsl]
                            )
                            xt.append(t_)
                        cq = s1.tile([128, CH], FP32, tag="cq")
                        sq = s1.tile([128, CH], FP32, tag="sq")
                        nc.sync.dma_start(out=cq[:], in_=cosq[:, tsl])
                        nc.sync.dma_start(out=sq[:], in_=sinq[:, tsl])
                        ksl = slice(c * NSUB * HD, (c + 1) * NSUB * HD)
                        cosk_t = s1.tile([128, NSUB * HD], FP32, tag="ck")
                        sink_t = s1.tile([128, NSUB * HD], FP32, tag="sk")
                        nc.sync.dma_start(out=cosk_t[:], in_=cosk[:, ksl])
                        nc.sync.dma_start(out=sink_t[:], in_=sink[:, ksl])

                        # ---- q path: (d, t) layout, one tile per head pair ----
                        for ct in range(NPAIR):
                            pq = ps1.tile([128, CH], FP32, tag="pq", bufs=2)
                            for dc in range(DC):
                                nc.tensor.matmul(
                                    pq[:],
                                    wq_t[dc][:, ct * 128 : (ct + 1) * 128],
                                    xt[dc][:],
                                    start=(dc == 0),
                                    stop=(dc == DC - 1),
                                )
                            sh = s1.tile([128, CH], FP32, tag="sh")
                            nc.vector.stream_shuffle(sh[:], pq[:], _SHUF_MASK)
                            tmp = s1.tile([128, CH], FP32, tag="tmp")
                            nc.vector.tensor_mul(tmp[:], pq[:], cq[:])
                            u = s1.tile([128, CH], FP32, tag="sh")
                            nc.vector.tensor_mul(u[:], sh[:], sq[:])
                            ro = s1.tile([128, CH], FP32, tag="tmp")
                            nc.gpsimd.tensor_add(ro[:], tmp[:], u[:])
                            m = s1.tile([128, CH], FP32, tag="sh")
                            nc.gpsimd.tensor_scalar_min(m[:], ro[:], 0.0)
                            e = s1.tile([128, CH], FP32, tag="tmp")
                            nc.scalar.activation(e[:], m[:], AF.Exp)
                            # qf = max(ro, 0) + e  == elu(ro)+1
                            nc.vector.scalar_tensor_tensor(
                                qf[ct][:, tsl],
                                in0=ro[:],
                                scalar=0.0,
                                in1=e[:],
                                op0=ALU.max,
                                op1=ALU.add,
                            )

                        # ---- k, v path: (t, d) layout per 128-token sub-chunk ----
                        for s4 in range(NSUB):
                            s = c * NSUB + s4
                            pk = ps1.tile([128, 512], FP32, tag="pk")
                            pv = ps1.tile([128, 512], FP32, tag="pv")
                            for dc in range(DC):
                                lhs = xt[dc][:, s4 * 128 : (s4 + 1) * 128]
                                nc.tensor.matmul(
                                    pk[:],
                                    lhs,
                                    wkv_t[dc][:, 0:512],
                                    start=(dc == 0),
                                    stop=(dc == DC - 1),
                                    skip_group_check=True,
                                )
                                nc.tensor.matmul(
                                    pv[:],
                                    lhs,
                                    wkv_t[dc][:, 512:1024],
                                    start=(dc == 0),
                                    stop=(dc == DC - 1),
                                    skip_group_check=True,
                                )
                            # v_ext = [v(heads 0-3) | 1 | v(heads 4-7) | 1]
                            vx = s1.tile([128, 516], F16, tag="vx")
                            nc.scalar.copy(
                                out=vx[:].rearrange("p (g c) -> p g c", g=2, c=258)[
                                    :, :, 0:256
                                ],
                                in_=pv[:].rearrange("p (g c) -> p g c", g=2, c=256),
                            )
                            nc.sync.dma_start(
                                out=vx[:].rearrange("p (g c) -> p g c", g=2, c=258)[
                                    :, :, 256:258
                                ],
                                in_=ones16[:].rearrange("p (g c) -> p g c", g=2),
                            )
                            # k rope in (t, d): interleaved pairs on free dim
                            ck = cosk_t[:, s4 * HD : (s4 + 1) * HD]
                            sk = sink_t[:, s4 * HD : (s4 + 1) * HD]
                            pk4 = pk[:].rearrange(
                                "p (h j two) -> p h j two", h=H_CORE, j=32, two=2
                            )
                            tmpk = s1.tile([128, 512], FP32, tag="tmpk")
                            nc.vector.tensor_mul(
                                tmpk[:].rearrange("p (h d) -> p h d", h=H_CORE),
                                pk[:].rearrange("p (h d) -> p h d", h=H_CORE),
                                ck.unsqueeze(1).broadcast_to([128, H_CORE, HD]),
                            )
                            uk = s1.tile([128, 512], FP32, tag="uk")
                            uk4 = uk[:].rearrange(
                                "p (h j two) -> p h j two", h=H_CORE, j=32, two=2
                            )
                            sk4 = sk.rearrange("p (j two) -> p j two", j=32)
                            for ev in range(2):
                                nc.vector.tensor_mul(
                                    uk4[:, :, :, ev],
                                    pk4[:, :, :, 1 - ev],
                                    sk4[:, :, ev : ev + 1]
                                    .unsqueeze(1)
                                    .broadcast_to([128, H_CORE, 32, 1])[:, :, :, 0],
                                )
                            rok = s1.tile([128, 512], FP32, tag="rok")
                            nc.gpsimd.tensor_add(rok[:], tmpk[:], uk[:])
                            mk = s1.tile([128, 512], FP32, tag="tmpk")
                            nc.gpsimd.tensor_scalar_min(mk[:], rok[:], 0.0)
                            ek = s1.tile([128, 512], FP32, tag="uk")
                            nc.scalar.activation(ek[:], mk[:], AF.Exp)
                            kf = s1.tile([128, 512], F16, tag="kf")
                            nc.vector.scalar_tensor_tensor(
                                kf[:],
                                in0=rok[:],
                                scalar=0.0,
                                in1=ek[:],
                                op0=ALU.max,
                                op1=ALU.add,
                            )
                            # kv state accumulation (+ ksum via the ones column)
                            for p in range(NPAIR):
                                rhs = (
                                    vx[:, 0:258] if p < 2 else vx[:, 258:516]
                                )
                                nc.tensor.matmul(
                                    kvps[p][:],
                                    kf[:, p * 128 : (p + 1) * 128],
                                    rhs,
                                    start=(s == 0),
                                    stop=(s == 32 - 1),
                                    skip_group_check=True,
                                )

                # ---- build stage-2 stationaries from kv state ----
                for p in range(NPAIR):
                    cA = (p % 2) * 128
                    nc.sync.dma_start(out=bdiag[p][:], in_=zpad[:].bitcast(FP32R))
                    nc.vector.tensor_copy(
                        out=bdiag[p][0:64, 0:64], in_=kvps[p][0:64, cA : cA + 64]
                    )
                    nc.vector.tensor_copy(
                        out=bdiag[p][64:128, 64:128],
                        in_=kvps[p][64:128, cA + 64 : cA + 128],
                    )
                    nc.sync.dma_start(
                        out=den_l[p][:], in_=zpad[:, 0:8].bitcast(FP32R)
                    )
                    nc.vector.tensor_copy(
                        out=den_l[p][0:64, 2 * p : 2 * p + 1],
                        in_=kvps[p][0:64, 256:257],
                    )
                    nc.vector.tensor_copy(
                        out=den_l[p][64:128, 2 * p + 1 : 2 * p + 2],
                        in_=kvps[p][64:128, 256:257],
                    )

            # ---------------- phase 2 ----------------
            with tc.tile_pool(name="w2", bufs=1) as w2, tc.tile_pool(
                name="s2", bufs=2
            ) as s2, tc.tile_pool(name="ps2", bufs=1, space="PSUM") as ps2:

                for c in range(NCH):
                    tsl = slice(c * CH, (c + 1) * CH)
                    # denominators for all 8 heads: (8, t)
                    dps = ps2.tile([8, CH], FP32, tag="dps", bufs=2)
                    for p in range(NPAIR):
                        nc.tensor.matmul(
                            dps[:],
                            den_l[p][:],
                            qf[p][:, tsl],
                            start=(p == 0),
                            stop=(p == NPAIR - 1),
                            skip_group_check=True,
                        )
                    dmax = s2.tile([8, CH], FP32, tag="dmax")
                    nc.vector.tensor_scalar_max(dmax[:], dps[:], 1e-6)
                    rscr = s2.tile([8, CH], FP32, tag="rscr")
                    rc32 = s2.tile([8, CH], FP32, tag="rc32")
                    nc.vector.reciprocal_approx_accurate(
                        out=rc32[:], in_=dmax[:], scratch=rscr[:]
                    )
                    rc = s2.tile([8, CH], FP32R, tag="rc")
                    nc.vector.tensor_copy(out=rc[:], in_=rc32[:])
                    rb = []
                    for p in range(NPAIR):
                        rb_ps = ps2.tile([128, CH], FP32, tag="rbps", bufs=2)
                        nc.tensor.matmul(
                            rb_ps[:],
                            sel_t[:, p * 128 : (p + 1) * 128],
                            rc[:],
                            start=True,
                            stop=True,
                        )
                        rbt = s2.tile([128, CH], FP32, tag=f"rb{p}")
                        nc.scalar.copy(out=rbt[:], in_=rb_ps[:])
                        rb.append(rbt)
                    asc = []
                    for p in range(NPAIR):
                        aps = ps2.tile([128, CH], FP32, tag="aps", bufs=2)
                        nc.tensor.matmul(
                            aps[:], bdiag[p][:], qf[p][:, tsl], start=True, stop=True
                        )
                        at = s2.tile([128, CH], F16, tag=f"asc{p}")
                        nc.vector.tensor_mul(at[:], aps[:], rb[p][:])
                        asc.append(at)
                    for do in range(8):
                        eps = ps2.tile([128, CH], FP32, tag="eps", bufs=2)
                        for p in range(NPAIR):
                            nc.tensor.matmul(
                                eps[:],
                                wo_t[p][:, do * 128 : (do + 1) * 128],
                                asc[p][:],
                                start=(p == 0),
                                stop=(p == NPAIR - 1),
                            )
                        ot = s2.tile([128, CH], FP32, tag="ot")
                        nc.scalar.copy(out=ot[:], in_=eps[:])
                        nc.sync.dma_start(
                            out=outT[do * 128 : (do + 1) * 128, tsl], in_=ot[:]
                        )

    nc.finalize()
    return nc


_NC = None


def _get_nc():
    global _NC
    if _NC is None:
        _NC = _build()
    return _NC


def _rope_tables():
    """Interleaved-order rope tables.

    orig head-dim d in [0,64); interleaved position: 2j <- d=j, 2j+1 <- d=j+32.
    rope(x)[d<32] = x[d] cos - x[d+32] sin ; [d>=32] = x[d] cos + x[d-32] sin
    After interleave + XOR-1 partner:
      out[2j]   = x[2j]  * cos_j - partner * sin_j   -> sinS[2j]   = -sin_j
      out[2j+1] = x[2j+1]* cos_j + partner * sin_j   -> sinS[2j+1] = +sin_j
    """
    j = np.arange(32, dtype=np.float64)
    inv_freq = ROPE_BASE ** (-2.0 * j / HD)
    t = np.arange(T, dtype=np.float64)
    ang = t[:, None] * inv_freq[None, :]  # (T, 32)
    cos = np.cos(ang)
    sin = np.sin(ang)
    cos_i = np.empty((T, HD), np.float64)
    sinS_i = np.empty((T, HD), np.float64)
    cos_i[:, 0::2] = cos
    cos_i[:, 1::2] = cos
    sinS_i[:, 0::2] = -sin
    sinS_i[:, 1::2] = sin
    return cos_i.astype(np.float32), sinS_i.astype(np.float32)


def _perm64():
    p = np.empty(HD, np.int64)
    j = np.arange(32)
    p[2 * j] = j
    p[2 * j + 1] = j + 32
    return p


def _prep_core_inputs(x, W_qkv, W_out):
    """Build the 8 per-core input maps."""
    B = x.shape[0]
    cos_i, sinS_i = _rope_tables()
    perm = _perm64()

    # (d,t)-layout q tables: stacked for the 2 heads of a pair, SCALE folded in
    cosq = np.concatenate([cos_i.T, cos_i.T], axis=0) * SCALE  # (128, T)
    sinq = np.concatenate([sinS_i.T, sinS_i.T], axis=0) * SCALE
    cosq = np.ascontiguousarray(cosq.astype(np.float32))
    sinq = np.ascontiguousarray(sinq.astype(np.float32))
    # (t,d)-layout k tables reshaped (128, 32*64): [p, s*64+d] = tab[s*128+p, d]
    cosk = np.ascontiguousarray(
        cos_i.reshape(32, 128, HD).transpose(1, 0, 2).reshape(128, 32 * HD)
    )
    sink = np.ascontiguousarray(
        sinS_i.reshape(32, 128, HD).transpose(1, 0, 2).reshape(128, 32 * HD)
    )

    sel_np = np.zeros((8, 512), np.float32)
    for p in range(4):
        sel_np[2 * p, p * 128 : p * 128 + 64] = 1.0
        sel_np[2 * p + 1, p * 128 + 64 : p * 128 + 128] = 1.0

    in_maps = []
    for core in range(8):
        b, g = divmod(core, 2)
        h0 = g * H_CORE
        qcols = np.concatenate(
            [(h0 + h) * HD + perm for h in range(H_CORE)]
        )  # interleaved q columns
        kcols = 1024 + qcols
        vcols = 2048 + np.arange(h0 * HD, h0 * HD + 512)
        wq_h = np.ascontiguousarray(W_qkv[:, qcols]).astype(np.float16)
        wkv_h = np.ascontiguousarray(
            np.concatenate([W_qkv[:, kcols], W_qkv[:, vcols]], axis=1)
        ).astype(np.float16)
        wo_h = np.ascontiguousarray(W_out[h0 * HD : h0 * HD + 512, :]).astype(np.float16)
        xT_b = np.ascontiguousarray(x[b].T).astype(np.float16)
        in_maps.append(
            {
                "xT": xT_b,
                "wq": wq_h,
                "wkv": wkv_h,
                "wo": wo_h,
                "cosq": cosq,
                "sinq": sinq,
                "cosk": cosk,
                "sink": sink,
                "sel": sel_np,
                "ones16": np.ones((128, 4), np.float16),
                "zpad": np.zeros((128, 128), np.float32),
            }
        )
    return in_maps


def kernel(x, W_qkv, W_out):
    x = np.asarray(x, dtype=np.float32)
    W_qkv = np.asarray(W_qkv, dtype=np.float32)
    W_out = np.asarray(W_out, dtype=np.float32)
    B = x.shape[0]

    nc = _get_nc()
    in_maps = _prep_core_inputs(x, W_qkv, W_out)
    res = run_bass_kernel_spmd(nc, in_maps, core_ids=list(range(8)))

    out = np.empty((B, T, DIM), np.float32)
    for b in range(B):
        acc = res.results[2 * b]["outT"] + res.results[2 * b + 1]["outT"]
        out[b] = acc.T
    return out
